# revision 10
# baseline (speedup 1.0000x reference)
"""Trainium2 Bass kernel for a char-LSTM (nn_CharsLstm).

Reference computation (B=4096 words, T=30 chars, D=512 emb, H=1024 hidden,
V=128 chars):
    xe = emb[x]                        # [B, T, D]
    scan over t: gates = xt @ W_ih.T + b_ih + h @ W_hh.T + b_hh
                 i, f, g, o = split(gates, 4)
                 c = sig(f)*c + sig(i)*tanh(g); h = sig(o)*tanh(c)
    return h                           # [B, H]

Strategy:
  - Data parallel: batch 4096 -> 8 cores x 512 words. No collectives.
  - Host folds embedding + input projection + both biases into one table:
        Wc = W_ih @ emb.T + (b_ih + b_hh)[:, None]    # [4H, V] = [4096, 128]
    so the x-path per step is a one-hot matmul with K=V=128 and the bias
    comes for free (each one-hot column sums to 1).
  - Everything is kept transposed on-chip (batch in the free dim):
    gates.T [4H, 512], h.T/c.T [H, 512]; elementwise produces h.T chunks
    [128, 512] exactly in the layout the next step's matmul needs.
  - h-part in fp8(e4m3) with MatmulPerfMode.DoubleRow: 2 weights per PE
    cell -> K=256 per matmul, halving the recurrent matmul count
    (256 -> 128 MMs/step). Weights are pre-scaled x16 on the host so the
    +-1/32 W_hh values use fp8 normal range; the activation stage applies
    the compensating 1/16 via its free input scale. x-part stays fp16.
    Emulated end-to-end rel err ~6.2e-3 (vs 2e-2 gate).
  - Gate blocks reordered (i,f,o,g) on the host, and the g-gate weights
    carry an extra x2 so tanh(g) = 2*sig(2g)-1: ONE sigmoid instruction
    covers all four gates [128, 2048] contiguous in PSUM (2 ACT instrs per
    chunk instead of 5), with the linear fix-up on the underloaded DVE.
    The last two chunks per step instead split the activations tanh-first
    to shorten the cross-step dependency tail. c-state and gate
    intermediates in fp16 (2x DVE modes); error contribution negligible.
  - Per step: 8 output chunks x (4 fp16 x-MMs + 16 fp8-DR MMs) = 160 MMs
    vs 288 fp16 MMs before.
"""

import numpy as np
import ml_dtypes

import concourse.bacc as bacc
import concourse.mybir as mybir
import concourse.tile as tile
from concourse.bass_utils import run_bass_kernel_spmd

B, T, D, H, V = 4096, 30, 512, 1024, 128
NCORES = 8
N = B // NCORES          # batch per core (matmul moving free dim)
KC = H // 128            # 8 h-chunks of 128
KC2 = KC // 2            # 4 DoubleRow chunks of 256
F32 = mybir.dt.float32
F16 = mybir.dt.float16
F8 = mybir.dt.float8e4
NP16 = np.float16
NP8 = ml_dtypes.float8_e4m3
DR = mybir.MatmulPerfMode.DoubleRow
DRSWI = mybir.MatmulPerfMode.DoubleRowSwInterleave
SIG = mybir.ActivationFunctionType.Sigmoid
TANH = mybir.ActivationFunctionType.Tanh
WSCALE = 16.0            # host pre-scale on all gate weights (fp8 range)
# SwInterleave: host pre-interleaves the DR weight pairs so LDWEIGHTS reads
# contiguously (FWL-compatible) instead of the HW interleave pattern, which
# disables FWL and leaves the 256-col weight load exposed (~213ns) in the
# fast-clock regime where the matmul itself is ~150ns.
SWI = True

_cached = {}


def build_kernel(n_steps=T, repeat=1, dma_split=8):
    nc = bacc.Bacc("TRN2", target_bir_lowering=False)

    # Host-prepared layouts (gate blocks reordered to i,f,o,g; weights x16):
    #  whh  [128, KC*4096] fp8  : whh[p, k*4096+m] = 16*W_hh[perm[m], k*128+p]
    #  wemb [128, 4096]    fp16 : wemb[v, m] = 16*Wc[perm[m], v]
    #  oh   [128, T*512]   fp16 : oh[v, t*512+b] = (x[b, t] == v)
    #  h0t  [128, KC*512]  fp8  : h0t[p, k*512+b] = h0[b, k*128+p]
    #  c0t  [128, KC*512]  fp16 : same layout as h0t
    #  out  [128, KC*512]  f32  : same layout (host inverts)
    whh_d = nc.dram_tensor("whh", [128, KC * 4096], F8, kind="ExternalInput")
    wemb_d = nc.dram_tensor("wemb", [128, 4 * H], F16, kind="ExternalInput")
    oh_d = nc.dram_tensor("oh", [128, n_steps * N], F16, kind="ExternalInput")
    h0_d = nc.dram_tensor("h0t", [128, KC * N], F8, kind="ExternalInput")
    c0_d = nc.dram_tensor("c0t", [128, KC * N], F16, kind="ExternalInput")
    out_d = nc.dram_tensor("out", [128, KC * N], F32, kind="ExternalOutput")

    inv = 1.0 / WSCALE

    with tile.TileContext(nc) as tc:
        with (
            tc.tile_pool(name="weights", bufs=1) as wpool,
            tc.tile_pool(name="state", bufs=2) as spool,
            tc.tile_pool(name="tmps", bufs=3) as tpool,
            tc.tile_pool(name="psum", bufs=2, space="PSUM") as ppool,
        ):
            # DMA emission order = consumption order: step-0 x-part needs
            # wemb + oh[0]; the DR k-loop needs h0 + whh chunks in k order;
            # ct is consumed by the first elementwise.
            # DMA order tuned so the PE unblocks ASAP: wemb + the first oh
            # slice feed step-0 x-MMs; h0 + the first whh chunks feed the DR
            # k-loop; ct (first elementwise) and the bulk of oh come after.
            wemb = wpool.tile([128, 4 * H], F16, tag="wemb")
            nc.sync.dma_start(out=wemb, in_=wemb_d[:, :])
            oh = wpool.tile([128, n_steps * N], F16, tag="oh")
            ohw = n_steps * N // 4
            nc.sync.dma_start(out=oh[:, 0:ohw], in_=oh_d[:, 0:ohw])
            ht = spool.tile([128, KC * N], F8, tag="ht")
            nc.sync.dma_start(out=ht, in_=h0_d[:, :])
            whh = wpool.tile([128, KC * 4096], F8, tag="whh")
            whw = KC * 4096 // dma_split
            for k in range(2):
                nc.sync.dma_start(out=whh[:, k * whw:(k + 1) * whw],
                                  in_=whh_d[:, k * whw:(k + 1) * whw])
            ct = wpool.tile([128, KC * N], F16, tag="ct")
            nc.sync.dma_start(out=ct, in_=c0_d[:, :])
            for k in range(2, dma_split):
                nc.sync.dma_start(out=whh[:, k * whw:(k + 1) * whw],
                                  in_=whh_d[:, k * whw:(k + 1) * whw])
            for k in range(1, 4):
                nc.sync.dma_start(out=oh[:, k * ohw:(k + 1) * ohw],
                                  in_=oh_d[:, k * ohw:(k + 1) * ohw])

            ht_fin = wpool.tile([128, KC * N], F32, tag="ht_fin")

            total = n_steps * repeat
            for s in range(total):
                t = s % n_steps
                last = s == total - 1
                ht_next = None if last else spool.tile([128, KC * N], F8, tag="ht")

                for j in range(KC):
                    # one PSUM tile = 4 banks = the 4 gate slices (i,f,o,g)
                    P = ppool.tile([128, 4 * N], F32, tag="ps",
                                   name=f"ps_{s}_{j}")
                    # x-part first (needs only wemb/oh), then DR chunks in k
                    # order: the first MM that needs h[k=6,7] (produced by the
                    # previous step's last elementwise chunk) comes 16 MMs in,
                    # so the cross-step serial tail hides under issued PE work.
                    for gi in range(4):
                        m0 = gi * H + j * 128
                        nc.tensor.matmul(
                            P[:, gi * N:(gi + 1) * N], wemb[:, m0:m0 + 128],
                            oh[:, t * N:(t + 1) * N],
                            start=True, stop=False,
                        )
                    for kk in range(KC2):
                        wsl = None if SWI else whh[
                            :, kk * 8192:(kk + 1) * 8192].rearrange(
                            'p (two m) -> p two m', two=2)
                        msl = ht[:, 2 * kk * N:(2 * kk + 2) * N].rearrange(
                            'p (two n) -> p two n', two=2)
                        for gi in range(4):
                            m0 = gi * H + j * 128
                            if SWI:
                                blk = kk * 32 + gi * KC + j
                                lhsT = whh[:, blk * 256:(blk + 1) * 256]
                            else:
                                lhsT = wsl[:, :, m0:m0 + 128]
                            nc.tensor.matmul(
                                P[:, gi * N:(gi + 1) * N], lhsT, msl,
                                start=False, stop=(kk == KC2 - 1),
                                perf_mode=DRSWI if SWI else DR,
                            )

                    # elementwise: PSUM gate order is (i, f, o, g); g-gate
                    # weights carry an extra x2 so tanh(g) = 2*sig(2g)-1 and
                    # one sigmoid instruction covers all four gates.
                    s_ifo = tpool.tile([128, 4 * N], F16, tag="s_ifo")
                    s_g = tpool.tile([128, N], F16, tag="s_g")
                    if j >= KC - 2:
                        # tail chunks feed the next step's last DR matmuls:
                        # split the sigmoid and order tanh(g) first so the
                        # c-chain starts ~1us earlier, shortening the
                        # cross-step PE stall. (tanh scale undoes the x2.)
                        nc.scalar.activation(out=s_g, in_=P[:, 3 * N:4 * N],
                                             func=TANH, scale=inv / 2)
                        nc.scalar.activation(out=s_ifo[:, 0:2 * N],
                                             in_=P[:, 0:2 * N],
                                             func=SIG, scale=inv)
                        nc.scalar.activation(out=s_ifo[:, 2 * N:3 * N],
                                             in_=P[:, 2 * N:3 * N],
                                             func=SIG, scale=inv)
                    else:
                        nc.scalar.activation(out=s_ifo, in_=P[:, 0:4 * N],
                                             func=SIG, scale=inv)
                        nc.vector.tensor_scalar(
                            out=s_g, in0=s_ifo[:, 3 * N:4 * N],
                            scalar1=2.0, scalar2=1.0,
                            op0=mybir.AluOpType.mult,
                            op1=mybir.AluOpType.subtract)  # 2*sig(2g)-1
                    tp = tpool.tile([128, N], F16, tag="tp")
                    nc.vector.tensor_mul(tp, s_ifo[:, 0:N], s_g)  # sig(i)*tanh(g)
                    c_sl = ct[:, j * N:(j + 1) * N]
                    nc.vector.tensor_mul(c_sl, c_sl, s_ifo[:, N:2 * N])
                    nc.vector.tensor_add(c_sl, c_sl, tp)          # c_new
                    s_tc = tpool.tile([128, N], F16, tag="s_tc")
                    nc.scalar.activation(out=s_tc, in_=c_sl, func=TANH)
                    h_sl = (ht_fin if last else ht_next)[:, j * N:(j + 1) * N]
                    nc.vector.tensor_mul(h_sl, s_ifo[:, 2 * N:3 * N], s_tc)
                    if last:
                        # stream each finished chunk out while the remaining
                        # chunks still compute
                        nc.sync.dma_start(out=out_d[:, j * N:(j + 1) * N],
                                          in_=h_sl)
                ht = ht_next

    nc.compile()
    return nc


# Gate-block permutation: torch order (i,f,g,o) -> kernel order (i,f,o,g)
_PERM = np.concatenate([
    np.arange(0, H), np.arange(H, 2 * H),
    np.arange(3 * H, 4 * H), np.arange(2 * H, 3 * H),
])


_GSCALE = np.ones((4 * H, 1), np.float32)
_GSCALE[3 * H:] = 2.0        # g-gate rows carry an extra x2 (tanh-via-sig)


def _prep_weights(emb, W_ih, W_hh, b_ih, b_hh):
    wc = W_ih @ emb.T + (b_ih + b_hh)[:, None]           # [4H, V]
    wc = wc[_PERM] * (WSCALE * _GSCALE)
    wemb_t = np.ascontiguousarray(wc.T).astype(NP16)     # [V, 4H]
    whh = W_hh[_PERM] * (WSCALE * _GSCALE)
    if SWI:
        # SwInterleave block layout: 256-col block per (kk, mchunk), where
        # flat[2*(127-r)+i] = whh[mchunk*128+r, (2kk+i)*128+p]  (the layout
        # bass_interp's DoubleRowSwInterleave deinterleave+reverse expects).
        a = whh.reshape(32, 128, KC, 128)        # [mchunk, r, k, p]
        t = a.transpose(3, 2, 0, 1)              # [p, k, mchunk, r]
        t = t.reshape(128, KC2, 2, 32, 128)      # [p, kk, i, mchunk, r]
        t = t[..., ::-1]                         # r -> c = 127-r
        t = t.transpose(0, 1, 3, 4, 2)           # [p, kk, mchunk, c, i]
        whh_t = np.ascontiguousarray(
            t.reshape(128, KC * 4096)).astype(NP8)
    else:
        # whh_t[p, k*4096+m] = whh[m, k*128+p]
        whh_t = np.ascontiguousarray(
            whh.T.reshape(KC, 128, 4 * H).transpose(1, 0, 2)
            .reshape(128, KC * 4096)
        ).astype(NP8)
    return wemb_t, whh_t


def _prep_core_inputs(x, whh_t, wemb_t, h0, c0, core, n_steps=T):
    sl = slice(core * N, (core + 1) * N)
    x_c = np.asarray(x[sl])                      # [N, T] ints
    oh = (np.arange(V, dtype=np.int64)[:, None, None]
          == x_c.T[None, :n_steps, :])           # [V, T, N]
    oh = oh.reshape(V, n_steps * N).astype(NP16)
    h0t = np.ascontiguousarray(
        h0[sl].reshape(N, KC, 128).transpose(2, 1, 0).reshape(128, KC * N)
    ).astype(NP8)
    c0t = np.ascontiguousarray(
        c0[sl].reshape(N, KC, 128).transpose(2, 1, 0).reshape(128, KC * N)
    ).astype(NP16)
    return {"whh": whh_t, "wemb": wemb_t, "oh": oh, "h0t": h0t, "c0t": c0t}


def kernel(x, emb, W_ih, W_hh, b_ih, b_hh, h0, c0, n_steps=T):
    x = np.asarray(x)
    emb = np.asarray(emb, dtype=np.float32)
    W_ih = np.asarray(W_ih, dtype=np.float32)
    W_hh = np.asarray(W_hh, dtype=np.float32)
    b_ih = np.asarray(b_ih, dtype=np.float32)
    b_hh = np.asarray(b_hh, dtype=np.float32)
    h0 = np.asarray(h0, dtype=np.float32)
    c0 = np.asarray(c0, dtype=np.float32)

    wemb_t, whh_t = _prep_weights(emb, W_ih, W_hh, b_ih, b_hh)

    key = n_steps
    if key not in _cached:
        _cached[key] = build_kernel(n_steps)
    nc = _cached[key]

    in_maps = [
        _prep_core_inputs(x, whh_t, wemb_t, h0, c0, core, n_steps)
        for core in range(NCORES)
    ]
    res = run_bass_kernel_spmd(nc, in_maps, core_ids=list(range(NCORES)))
    kernel.last_results = res

    out = np.empty((B, H), dtype=np.float32)
    for core in range(NCORES):
        ot = res.results[core]["out"]                    # [128, KC*N]
        out[core * N:(core + 1) * N] = (
            ot.reshape(128, KC, N).transpose(2, 1, 0).reshape(N, H)
        )
    return out


# revision 12
# speedup vs baseline: 1.0084x; 1.0084x over previous
"""Trainium2 Bass kernel for a char-LSTM (nn_CharsLstm).

Reference computation (B=4096 words, T=30 chars, D=512 emb, H=1024 hidden,
V=128 chars):
    xe = emb[x]                        # [B, T, D]
    scan over t: gates = xt @ W_ih.T + b_ih + h @ W_hh.T + b_hh
                 i, f, g, o = split(gates, 4)
                 c = sig(f)*c + sig(i)*tanh(g); h = sig(o)*tanh(c)
    return h                           # [B, H]

Strategy:
  - Data parallel: batch 4096 -> 8 cores x 512 words. No collectives.
  - Host folds embedding + input projection + both biases into one table:
        Wc = W_ih @ emb.T + (b_ih + b_hh)[:, None]    # [4H, V] = [4096, 128]
    so the x-path per step is a one-hot matmul with K=V=128 and the bias
    comes for free (each one-hot column sums to 1).
  - Everything is kept transposed on-chip (batch in the free dim):
    gates.T [4H, 512], h.T/c.T [H, 512]; elementwise produces h.T chunks
    [128, 512] exactly in the layout the next step's matmul needs.
  - h-part in fp8(e4m3) with MatmulPerfMode.DoubleRowSwInterleave: 2
    weights per PE cell -> K=256 per matmul, halving the recurrent matmul
    count (256 -> 128 MMs/step). The host pre-interleaves the weight pairs
    (SWI flag) so LDWEIGHTS reads contiguously instead of the HW interleave
    pattern that disables fast weight load. Weights are pre-scaled x16 on
    the host so the +-1/32 W_hh values use fp8 normal range; the activation
    stage applies the compensating 1/16 via its free input scale. x-part
    stays fp16. Emulated end-to-end rel err ~6.2e-3 (vs 2e-2 gate).
  - Gate blocks reordered (i,f,o,g) on the host, and the g-gate weights
    carry an extra x2 so tanh(g) = 2*sig(2g)-1: ONE sigmoid instruction
    covers all four gates [128, 2048] contiguous in PSUM (2 ACT instrs per
    chunk instead of 5), with the linear fix-up on the underloaded DVE.
    The last two chunks per step instead split the activations tanh-first
    to shorten the cross-step dependency tail. c-state and gate
    intermediates in fp16 (2x DVE modes); error contribution negligible.
  - Per step: 8 output chunks x (4 fp16 x-MMs + 16 fp8-DR MMs) = 160 MMs
    vs 288 fp16 MMs before.
"""

import numpy as np
import ml_dtypes

import concourse.bacc as bacc
import concourse.mybir as mybir
import concourse.tile as tile
from concourse.bass_utils import run_bass_kernel_spmd

B, T, D, H, V = 4096, 30, 512, 1024, 128
NCORES = 8
N = B // NCORES          # batch per core (matmul moving free dim)
KC = H // 128            # 8 h-chunks of 128
KC2 = KC // 2            # 4 DoubleRow chunks of 256
F32 = mybir.dt.float32
F16 = mybir.dt.float16
F8 = mybir.dt.float8e4
NP16 = np.float16
NP8 = ml_dtypes.float8_e4m3
DR = mybir.MatmulPerfMode.DoubleRow
DRSWI = mybir.MatmulPerfMode.DoubleRowSwInterleave
SIG = mybir.ActivationFunctionType.Sigmoid
TANH = mybir.ActivationFunctionType.Tanh
WSCALE = 16.0            # host pre-scale on all gate weights (fp8 range)
# SwInterleave: host pre-interleaves the DR weight pairs so LDWEIGHTS reads
# contiguously (FWL-compatible) instead of the HW interleave pattern, which
# disables FWL and leaves the 256-col weight load exposed (~213ns) in the
# fast-clock regime where the matmul itself is ~150ns.
SWI = True

_cached = {}


def build_kernel(n_steps=T, repeat=1, dma_split=8):
    nc = bacc.Bacc("TRN2", target_bir_lowering=False)

    # Host-prepared layouts (gate blocks reordered to i,f,o,g; weights x16):
    #  whh  [128, KC*4096] fp8  : whh[p, k*4096+m] = 16*W_hh[perm[m], k*128+p]
    #  wemb [128, 4096]    fp16 : wemb[v, m] = 16*Wc[perm[m], v]
    #  oh   [128, T*512]   fp16 : oh[v, t*512+b] = (x[b, t] == v)
    #  h0t  [128, KC*512]  fp8  : h0t[p, k*512+b] = h0[b, k*128+p]
    #  c0t  [128, KC*512]  fp16 : same layout as h0t
    #  out  [128, KC*512]  f32  : same layout (host inverts)
    whh_d = nc.dram_tensor("whh", [128, KC * 4096], F8, kind="ExternalInput")
    wemb_d = nc.dram_tensor("wemb", [128, 4 * H], F16, kind="ExternalInput")
    oh_d = nc.dram_tensor("oh", [128, n_steps * N], F16, kind="ExternalInput")
    h0_d = nc.dram_tensor("h0t", [128, KC * N], F8, kind="ExternalInput")
    c0_d = nc.dram_tensor("c0t", [128, KC * N], F16, kind="ExternalInput")
    out_d = nc.dram_tensor("out", [128, KC * N], F32, kind="ExternalOutput")

    inv = 1.0 / WSCALE

    with tile.TileContext(nc) as tc:
        with (
            tc.tile_pool(name="weights", bufs=1) as wpool,
            tc.tile_pool(name="state", bufs=2) as spool,
            tc.tile_pool(name="tmps", bufs=3) as tpool,
            tc.tile_pool(name="psum", bufs=2, space="PSUM") as ppool,
        ):
            # DMA emission order = consumption order: step-0 x-part needs
            # wemb + oh[0]; the DR k-loop needs h0 + whh chunks in k order;
            # ct is consumed by the first elementwise.
            # DMA order tuned so the PE unblocks ASAP: wemb + the first oh
            # slice feed step-0 x-MMs; h0 + the first whh chunks feed the DR
            # k-loop; ct (first elementwise) and the bulk of oh come after.
            wemb = wpool.tile([128, 4 * H], F16, tag="wemb")
            nc.sync.dma_start(out=wemb, in_=wemb_d[:, :])
            oh = wpool.tile([128, n_steps * N], F16, tag="oh")
            ohw = n_steps * N // 4
            nc.sync.dma_start(out=oh[:, 0:ohw], in_=oh_d[:, 0:ohw])
            ht = spool.tile([128, KC * N], F8, tag="ht")
            nc.sync.dma_start(out=ht, in_=h0_d[:, :])
            whh = wpool.tile([128, KC * 4096], F8, tag="whh")
            whw = KC * 4096 // dma_split
            for k in range(2):
                nc.sync.dma_start(out=whh[:, k * whw:(k + 1) * whw],
                                  in_=whh_d[:, k * whw:(k + 1) * whw])
            ct = wpool.tile([128, KC * N], F16, tag="ct")
            nc.sync.dma_start(out=ct, in_=c0_d[:, :])
            for k in range(2, dma_split):
                nc.sync.dma_start(out=whh[:, k * whw:(k + 1) * whw],
                                  in_=whh_d[:, k * whw:(k + 1) * whw])
            for k in range(1, 4):
                nc.sync.dma_start(out=oh[:, k * ohw:(k + 1) * ohw],
                                  in_=oh_d[:, k * ohw:(k + 1) * ohw])

            ht_fin = wpool.tile([128, KC * N], F32, tag="ht_fin")

            total = n_steps * repeat
            for s in range(total):
                t = s % n_steps
                last = s == total - 1
                ht_next = None if last else spool.tile([128, KC * N], F8, tag="ht")

                for j in range(KC):
                    # one PSUM tile = 4 banks = the 4 gate slices (i,f,o,g)
                    P = ppool.tile([128, 4 * N], F32, tag="ps",
                                   name=f"ps_{s}_{j}")
                    # x-part first (needs only wemb/oh), then DR chunks in k
                    # order: the first MM that needs h[k=6,7] (produced by the
                    # previous step's last elementwise chunk) comes 16 MMs in,
                    # so the cross-step serial tail hides under issued PE work.
                    for gi in range(4):
                        m0 = gi * H + j * 128
                        nc.tensor.matmul(
                            P[:, gi * N:(gi + 1) * N], wemb[:, m0:m0 + 128],
                            oh[:, t * N:(t + 1) * N],
                            start=True, stop=False,
                        )
                    for kk in range(KC2):
                        wsl = None if SWI else whh[
                            :, kk * 8192:(kk + 1) * 8192].rearrange(
                            'p (two m) -> p two m', two=2)
                        msl = ht[:, 2 * kk * N:(2 * kk + 2) * N].rearrange(
                            'p (two n) -> p two n', two=2)
                        for gi in range(4):
                            m0 = gi * H + j * 128
                            if SWI:
                                blk = kk * 32 + gi * KC + j
                                lhsT = whh[:, blk * 256:(blk + 1) * 256]
                            else:
                                lhsT = wsl[:, :, m0:m0 + 128]
                            nc.tensor.matmul(
                                P[:, gi * N:(gi + 1) * N], lhsT, msl,
                                start=False, stop=(kk == KC2 - 1),
                                perf_mode=DRSWI if SWI else DR,
                            )

                    # elementwise: PSUM gate order is (i, f, o, g); g-gate
                    # weights carry an extra x2 so tanh(g) = 2*sig(2g)-1 and
                    # one sigmoid instruction covers all four gates.
                    s_ifo = tpool.tile([128, 4 * N], F16, tag="s_ifo")
                    s_g = tpool.tile([128, N], F16, tag="s_g")
                    if j >= KC - 1:
                        # the last chunk feeds the next step's final DR
                        # matmuls: split the sigmoid and order tanh(g) first
                        # so the c-chain starts ~1us earlier, shortening the
                        # cross-step PE stall. (tanh scale undoes the x2.)
                        # j=6 deliberately uses the fused path: fewer ACT
                        # inits lets the FIFO reach j=7's chain sooner.
                        nc.scalar.activation(out=s_g, in_=P[:, 3 * N:4 * N],
                                             func=TANH, scale=inv / 2)
                        nc.scalar.activation(out=s_ifo[:, 0:2 * N],
                                             in_=P[:, 0:2 * N],
                                             func=SIG, scale=inv)
                        nc.scalar.activation(out=s_ifo[:, 2 * N:3 * N],
                                             in_=P[:, 2 * N:3 * N],
                                             func=SIG, scale=inv)
                    else:
                        nc.scalar.activation(out=s_ifo, in_=P[:, 0:4 * N],
                                             func=SIG, scale=inv)
                        nc.vector.tensor_scalar(
                            out=s_g, in0=s_ifo[:, 3 * N:4 * N],
                            scalar1=2.0, scalar2=1.0,
                            op0=mybir.AluOpType.mult,
                            op1=mybir.AluOpType.subtract)  # 2*sig(2g)-1
                    tp = tpool.tile([128, N], F16, tag="tp")
                    nc.vector.tensor_mul(tp, s_ifo[:, 0:N], s_g)  # sig(i)*tanh(g)
                    c_sl = ct[:, j * N:(j + 1) * N]
                    nc.vector.tensor_mul(c_sl, c_sl, s_ifo[:, N:2 * N])
                    nc.vector.tensor_add(c_sl, c_sl, tp)          # c_new
                    s_tc = tpool.tile([128, N], F16, tag="s_tc")
                    nc.scalar.activation(out=s_tc, in_=c_sl, func=TANH)
                    h_sl = (ht_fin if last else ht_next)[:, j * N:(j + 1) * N]
                    nc.vector.tensor_mul(h_sl, s_ifo[:, 2 * N:3 * N], s_tc)
                    if last:
                        # stream each finished chunk out while the remaining
                        # chunks still compute
                        nc.sync.dma_start(out=out_d[:, j * N:(j + 1) * N],
                                          in_=h_sl)
                ht = ht_next

    nc.compile()
    return nc


# Gate-block permutation: torch order (i,f,g,o) -> kernel order (i,f,o,g)
_PERM = np.concatenate([
    np.arange(0, H), np.arange(H, 2 * H),
    np.arange(3 * H, 4 * H), np.arange(2 * H, 3 * H),
])


_GSCALE = np.ones((4 * H, 1), np.float32)
_GSCALE[3 * H:] = 2.0        # g-gate rows carry an extra x2 (tanh-via-sig)


def _prep_weights(emb, W_ih, W_hh, b_ih, b_hh):
    wc = W_ih @ emb.T + (b_ih + b_hh)[:, None]           # [4H, V]
    wc = wc[_PERM] * (WSCALE * _GSCALE)
    wemb_t = np.ascontiguousarray(wc.T).astype(NP16)     # [V, 4H]
    whh = W_hh[_PERM] * (WSCALE * _GSCALE)
    if SWI:
        # SwInterleave block layout: 256-col block per (kk, mchunk), where
        # flat[2*(127-r)+i] = whh[mchunk*128+r, (2kk+i)*128+p]  (the layout
        # bass_interp's DoubleRowSwInterleave deinterleave+reverse expects).
        a = whh.reshape(32, 128, KC, 128)        # [mchunk, r, k, p]
        t = a.transpose(3, 2, 0, 1)              # [p, k, mchunk, r]
        t = t.reshape(128, KC2, 2, 32, 128)      # [p, kk, i, mchunk, r]
        t = t[..., ::-1]                         # r -> c = 127-r
        t = t.transpose(0, 1, 3, 4, 2)           # [p, kk, mchunk, c, i]
        whh_t = np.ascontiguousarray(
            t.reshape(128, KC * 4096)).astype(NP8)
    else:
        # whh_t[p, k*4096+m] = whh[m, k*128+p]
        whh_t = np.ascontiguousarray(
            whh.T.reshape(KC, 128, 4 * H).transpose(1, 0, 2)
            .reshape(128, KC * 4096)
        ).astype(NP8)
    return wemb_t, whh_t


def _prep_core_inputs(x, whh_t, wemb_t, h0, c0, core, n_steps=T):
    sl = slice(core * N, (core + 1) * N)
    x_c = np.asarray(x[sl])                      # [N, T] ints
    oh = (np.arange(V, dtype=np.int64)[:, None, None]
          == x_c.T[None, :n_steps, :])           # [V, T, N]
    oh = oh.reshape(V, n_steps * N).astype(NP16)
    h0t = np.ascontiguousarray(
        h0[sl].reshape(N, KC, 128).transpose(2, 1, 0).reshape(128, KC * N)
    ).astype(NP8)
    c0t = np.ascontiguousarray(
        c0[sl].reshape(N, KC, 128).transpose(2, 1, 0).reshape(128, KC * N)
    ).astype(NP16)
    return {"whh": whh_t, "wemb": wemb_t, "oh": oh, "h0t": h0t, "c0t": c0t}


def kernel(x, emb, W_ih, W_hh, b_ih, b_hh, h0, c0, n_steps=T):
    x = np.asarray(x)
    emb = np.asarray(emb, dtype=np.float32)
    W_ih = np.asarray(W_ih, dtype=np.float32)
    W_hh = np.asarray(W_hh, dtype=np.float32)
    b_ih = np.asarray(b_ih, dtype=np.float32)
    b_hh = np.asarray(b_hh, dtype=np.float32)
    h0 = np.asarray(h0, dtype=np.float32)
    c0 = np.asarray(c0, dtype=np.float32)

    wemb_t, whh_t = _prep_weights(emb, W_ih, W_hh, b_ih, b_hh)

    key = n_steps
    if key not in _cached:
        _cached[key] = build_kernel(n_steps)
    nc = _cached[key]

    in_maps = [
        _prep_core_inputs(x, whh_t, wemb_t, h0, c0, core, n_steps)
        for core in range(NCORES)
    ]
    res = run_bass_kernel_spmd(nc, in_maps, core_ids=list(range(NCORES)))
    kernel.last_results = res

    out = np.empty((B, H), dtype=np.float32)
    for core in range(NCORES):
        ot = res.results[core]["out"]                    # [128, KC*N]
        out[core * N:(core + 1) * N] = (
            ot.reshape(128, KC, N).transpose(2, 1, 0).reshape(N, H)
        )
    return out
